# revision 20
# baseline (speedup 1.0000x reference)
"""Trainium2 Bass kernel for nn_AttentionModule (B=4, C=256, 64x64 spatial).

Reference computation (per batch b, x flattened to [C, HW]):
    q = Wq @ x + bq            [32, HW] -> per-pixel queries
    k = Wk @ x + bk            [32, HW]
    v = x^T @ Wv^T + bv        [HW, 256]
    out = softmax(q^T @ k) @ v [HW, 256] -> transposed to [C, HW]

Sharding: 8 cores, data-parallel over (batch, query-half): core = 2*b + h
computes queries [h*2048, (h+1)*2048) of batch b against all 4096 keys.
Weights replicated. The per-core q slice arrives as separate input data
(xq) so the program stays SPMD-identical.

Numerics: fp16 inputs/projections (5e-4 rounding), fp32 PSUM accumulate,
bf16 attention probabilities (fp16 would overflow: scores reach +-39).
Expected end-to-end ~5e-3 max-rel vs the fp32 reference.

Device layout:
  - scores computed transposed ([keys, q]) so the softmax denominator is
    accumulated by the PE itself: v carries ones columns, out[:, 256] =
    sum_k exp(s). exp on ScalarE straight out of PSUM, no max-subtraction
    (|s| <= ~40 is safe in fp32).
  - QK is 2-way row-packed: k tiles 0-15 live at partitions 0-31, tiles
    16-31 at partitions 32-63 (tile_position row groups), with q
    replicated to both blocks. Two K=32 matmuls run concurrently in the
    PE array; one [128, 1024] ACTIVATE converts both score tiles.
  - out tiles are [q, 258] in PSUM; normalization is per-partition
    reciprocal + tensor_scalar multiply on VectorE, fp32.
  - final [q, c] -> [c, q] transpose + bv bias happen host-side in the
    unshard step.
"""
import numpy as np
from contextlib import ExitStack

import concourse.bass as bass
import concourse.bacc as bacc
import concourse.tile as tile
from concourse import mybir
from concourse.bass_utils import run_bass_kernel_spmd

B, C, H, W = 4, 256, 64, 64
HW = H * W            # 4096
D = C // 8            # 32 (q/k channels)
NCORES = 8
Q = HW // 2           # 2048 queries per core
QC = 512              # q chunk (matmul moving dim)
NCH = Q // QC         # 4 chunks
KT = HW // 128        # 32 key tiles
P = 128
VW = C + 2            # v tile width (ones col + even-pad)

F32 = mybir.dt.float32
F16 = mybir.dt.float16
BF16 = mybir.dt.bfloat16
EXP = mybir.ActivationFunctionType.Exp

_CACHE: dict = {}


def build_program(with_bias: bool = False) -> bacc.Bacc:
    nc = bacc.Bacc("TRN2", target_bir_lowering=False, debug=False)

    xkv_d = nc.dram_tensor("xkv", [C, HW], F16, kind="ExternalInput").ap()
    xq_d = nc.dram_tensor("xq", [C, Q], F16, kind="ExternalInput").ap()
    # packed per c'-half: [wqT | wkT | wvT]  [256, 320]
    wpk_d = nc.dram_tensor("wpk", [C, 2 * D + C], F16, kind="ExternalInput").ap()
    # packed [bq | bk | ones(QC)]
    bpk_d = nc.dram_tensor("bpk", [1, 2 * D + QC], F16, kind="ExternalInput").ap()
    o_d = nc.dram_tensor("o", [Q, C], F16, kind="ExternalOutput").ap()

    with tile.TileContext(nc) as tc:
        with ExitStack() as ctx:
            big = ctx.enter_context(tc.tile_pool(name="big", bufs=24))
            const = ctx.enter_context(tc.tile_pool(name="const", bufs=1))
            ep = ctx.enter_context(tc.tile_pool(name="ep", bufs=4))
            ps = ctx.enter_context(tc.tile_pool(name="ps", bufs=2, space="PSUM"))
            po = ctx.enter_context(tc.tile_pool(name="po", bufs=4, space="PSUM"))

            # ---- PE warm-up: the HAM clock gate releases only under
            # sustained full-array activity; run dummy matmuls on a zeroed
            # tile while the input DMAs are still in flight ----
            dummy = const.tile([P, QC], F16, tag="dummy")
            nc.vector.memset(dummy[:], 0.0)
            wps = po.tile([P, QC], F32, tag="o", name="wps")
            for _ in range(8):
                nc.tensor.matmul(wps[:], dummy[:, 0:P], dummy[:],
                                 start=True, stop=True)

            # ---- constants / weights (3 triggers total) ----
            wpk_t = [const.tile([P, 2 * D + C], F16, tag=f"wpk{i}", name=f"wpk{i}")
                     for i in range(2)]
            for i in range(2):
                nc.scalar.dma_start(wpk_t[i][:], wpk_d[i * P:(i + 1) * P, :])
            bpk_t = const.tile([1, 2 * D + QC], F16, tag="bpk")
            nc.sync.dma_start(bpk_t[:], bpk_d)
            wq_sb = [wpk_t[i][:, 0:D] for i in range(2)]
            wk_sb = [wpk_t[i][:, D:2 * D] for i in range(2)]
            wv_sb = [wpk_t[i][:, 2 * D:] for i in range(2)]
            bq_sb = bpk_t[:, 0:D]
            bk_sb = bpk_t[:, D:2 * D]
            ones_sb = bpk_t[:, 2 * D:]

            # qrep: q^T replicated at partition blocks 0-31 and 32-63
            qrep = const.tile([2 * D, Q], F16, tag="qrep")
            # kT4: EVEN k tiles at partitions 0-31 (col (kt//2)*128),
            #      ODD  k tiles at partitions 32-63
            kT4 = const.tile([2 * D, 16 * P], F16, tag="kT4")
            kstage = const.tile([D, 16 * P], F16, tag="kstage")
            v_all = const.tile([P, KT * VW], F16, tag="vall")
            nc.vector.memset(
                v_all[:].rearrange("p (k c) -> p k c", c=VW)[:, :, C:C + 2], 1.0)

            # ---- x: two big const tiles per half, split across both HWDGE
            # queues (sync=SP, scalar=ACT; ACT is idle during the load) ----
            xkv_t = [const.tile([P, HW], F16, tag=f"xkv{i}", name=f"xkv{i}")
                     for i in range(2)]
            xq_t = [const.tile([P, Q], F16, tag=f"xq{i}", name=f"xq{i}")
                    for i in range(2)]
            eng = [nc.sync, nc.scalar]
            QT4 = HW // 4
            for quarter in (0, 1, 2, 3):
                for i in range(2):
                    eng[i].dma_start(
                        xkv_t[i][:, quarter * QT4:(quarter + 1) * QT4],
                        xkv_d[i * P:(i + 1) * P, quarter * QT4:(quarter + 1) * QT4])
                if quarter == 0:
                    for i in range(2):
                        eng[i].dma_start(xq_t[i][:, 0:QC], xq_d[i * P:(i + 1) * P, 0:QC])
            for i in range(2):
                eng[i].dma_start(xq_t[i][:, QC:], xq_d[i * P:(i + 1) * P, QC:])
            xq_sb = [[xq_t[i][:, j * QC:(j + 1) * QC] for j in range(Q // QC)]
                     for i in range(2)]
            xkv_sb = [[xkv_t[i][:, j * QC:(j + 1) * QC] for j in range(HW // QC)]
                      for i in range(2)]

            # ---- projections (PE, fp16 in / f32 psum) ----
            def kproj(j):
                kp = po.tile([D, QC], F32, tag="o", name="kp")
                nc.tensor.matmul(kp[:], wk_sb[0], xkv_sb[0][j],
                                 start=True, stop=not with_bias)
                nc.tensor.matmul(kp[:], wk_sb[1], xkv_sb[1][j],
                                 start=False, stop=not with_bias)
                if with_bias:
                    nc.tensor.matmul(kp[:], bk_sb, ones_sb, start=False, stop=True)
                # chunk j = k tiles 4j..4j+3: evens -> kT4[0:32], odds -> stage
                src = kp[:].rearrange("p (t c) -> p t c", c=P)
                dst_e = kT4[0:D, 2 * j * P:(2 * j + 2) * P].rearrange(
                    "p (t c) -> p t c", c=P)
                dst_o = kstage[:, 2 * j * P:(2 * j + 2) * P].rearrange(
                    "p (t c) -> p t c", c=P)
                nc.vector.tensor_copy(dst_o, src[:, 1::2])
                nc.vector.tensor_copy(dst_e, src[:, 0::2])
                nc.sync.dma_start(kT4[D:2 * D, 2 * j * P:(2 * j + 2) * P],
                                  kstage[:, 2 * j * P:(2 * j + 2) * P])

            def qproj(j):
                qp = po.tile([D, QC], F32, tag="o", name="qp")
                nc.tensor.matmul(qp[:], wq_sb[0], xq_sb[0][j],
                                 start=True, stop=not with_bias)
                nc.tensor.matmul(qp[:], wq_sb[1], xq_sb[1][j],
                                 start=False, stop=not with_bias)
                if with_bias:
                    nc.tensor.matmul(qp[:], bq_sb, ones_sb, start=False, stop=True)
                nc.vector.tensor_copy(qrep[0:D, j * QC:(j + 1) * QC], qp[:])
                nc.sync.dma_start(qrep[D:2 * D, j * QC:(j + 1) * QC],
                                  qrep[0:D, j * QC:(j + 1) * QC])

            v_sb = [v_all[:, t * VW:(t + 1) * VW] for t in range(KT)]

            def vproj(t):
                j, off = divmod(t, QC // P)
                vp = po.tile([P, C], F32, tag="o", name="vp")
                nc.tensor.matmul(
                    vp[:], xkv_sb[0][j][:, off * P:(off + 1) * P], wv_sb[0],
                    start=True, stop=False)
                nc.tensor.matmul(
                    vp[:], xkv_sb[1][j][:, off * P:(off + 1) * P], wv_sb[1],
                    start=False, stop=True)
                nc.vector.tensor_copy(v_sb[t][:, 0:C], vp[:])

            def vpair(g):
                vproj(2 * g)
                vproj(2 * g + 1)

            # chunk-0 deps first; the rest interleaves into the early
            # attention steps (all proj psum use ends before AV claims po).
            # With the even/odd pairing everything streams in natural order:
            # QK pair g needs kproj(g//2), AV pair g needs vpair(g).
            kproj(0)
            qproj(0)
            proj_work = [
                lambda: vpair(0), lambda: vpair(1),
                lambda: kproj(1), lambda: vpair(2),
                lambda: qproj(1), lambda: vpair(3),
                lambda: kproj(2), lambda: vpair(4), lambda: vpair(5),
                lambda: kproj(3), lambda: vpair(6),
                lambda: qproj(2), lambda: vpair(7),
                lambda: kproj(4), lambda: vpair(8), lambda: vpair(9),
                lambda: kproj(5), lambda: vpair(10),
                lambda: qproj(3), lambda: vpair(11),
                lambda: kproj(6), lambda: vpair(12), lambda: vpair(13),
                lambda: kproj(7), lambda: vpair(14), lambda: vpair(15),
            ]

            # ---- attention: flat 64-pair stream, AV lags QK by 2 pairs ----
            # pair (ci, g) = exp(scores) for k tiles (g, 16+g) of q chunk ci,
            # one [128, 1024] bf16 tile. AV of pair g covers kt=g and kt=16+g;
            # kt accumulation order [0,16,1,17,...] so start is kt==0 (pos 0)
            # and stop is kt==31 (pos 31).
            NP = NCH * 16
            AV_START = 18       # po banks stay proj-owned before this step

            def av_epilogue(ops, ci):
                for qs in range(QC // P):
                    op = ops[qs]
                    rinv = ep.tile([P, 1], F32, tag="rinv", name="rinv")
                    nc.vector.reciprocal(rinv[:], op[:, C:C + 1])
                    osb = ep.tile([P, C], F16, tag="osb", name="osb")
                    nc.vector.tensor_scalar_mul(osb[:], op[:, 0:C], rinv[:])
                    q0 = (ci * (QC // P) + qs) * P
                    nc.sync.dma_start(o_d[q0:q0 + P, :], osb[:])

            def av_pair(ops, pair_tile, g):
                for kt, half in ((2 * g, 0), (2 * g + 1, 1)):
                    for qs in range(QC // P):
                        nc.tensor.matmul(
                            ops[qs][:],
                            pair_tile[:, half * QC + qs * P: half * QC + (qs + 1) * P],
                            v_sb[kt][:],
                            start=(kt == 0), stop=(kt == KT - 1))

            pair_tiles = {}
            ops = {}
            av_done = 0
            step = 0
            wi = 0
            while av_done < NP:
                if step < NP:
                    ci, g = divmod(step, 16)
                    sc = ps.tile([P, 2 * QC], F32, tag="p", name="sc")
                    nc.tensor.matmul(
                        sc[:, 0:QC], kT4[0:D, g * P:(g + 1) * P],
                        qrep[0:D, ci * QC:(ci + 1) * QC],
                        start=True, stop=True, tile_position=(0, 0))
                    if step < 6:
                        # kT4 block 1 arrives via a staged DMA that rides
                        # behind the x-input queue; read the stage directly
                        # (partitions 0-31, unpacked) for the first pairs
                        nc.tensor.matmul(
                            sc[:, QC:2 * QC], kstage[:, g * P:(g + 1) * P],
                            qrep[0:D, ci * QC:(ci + 1) * QC],
                            start=True, stop=True, tile_position=(0, 0))
                    else:
                        nc.tensor.matmul(
                            sc[:, QC:2 * QC], kT4[D:2 * D, g * P:(g + 1) * P],
                            qrep[D:2 * D, ci * QC:(ci + 1) * QC],
                            start=True, stop=True, tile_position=(D, 0))
                    Pt = big.tile([P, 2 * QC], BF16, tag="big", name="pt")
                    nc.scalar.activation(Pt[:], sc[:], EXP)
                    pair_tiles[step] = Pt
                for _ in range(2):
                    if wi < len(proj_work):
                        proj_work[wi]()
                        wi += 1
                budget = 2 if step >= AV_START else 0
                while budget > 0 and av_done < NP and av_done <= step - 2:
                    cav, gav = divmod(av_done, 16)
                    if gav == 0:
                        ops[cav] = [po.tile([P, VW], F32, tag="o", name="avo")
                                    for _ in range(QC // P)]
                    av_pair(ops[cav], pair_tiles.pop(av_done), gav)
                    if gav == 15:
                        av_epilogue(ops.pop(cav), cav)
                    av_done += 1
                    budget -= 1
                step += 1

    nc.compile()
    return nc


def _in_maps(x, Wq, bq, Wk, bk, Wv, bv):
    xf = np.ascontiguousarray(np.asarray(x, np.float32).reshape(B, C, HW)).astype(np.float16)
    wpk = np.concatenate([
        np.asarray(Wq, np.float32).T,
        np.asarray(Wk, np.float32).T,
        np.asarray(Wv, np.float32).T], axis=1).astype(np.float16)
    bpk = np.concatenate([
        np.asarray(bq, np.float32).reshape(1, D),
        np.asarray(bk, np.float32).reshape(1, D),
        np.ones((1, QC), np.float32)], axis=1).astype(np.float16)
    maps = []
    for core in range(NCORES):
        b, h = divmod(core, 2)
        maps.append({
            "xkv": xf[b],
            "xq": np.ascontiguousarray(xf[b][:, h * Q:(h + 1) * Q]),
            "wpk": np.ascontiguousarray(wpk),
            "bpk": np.ascontiguousarray(bpk),
        })
    return maps


def _gather(results, bv):
    out = np.empty((B, C, HW), np.float32)
    for core in range(NCORES):
        b, h = divmod(core, 2)
        out[b][:, h * Q:(h + 1) * Q] = results[core]["o"].T
    out += np.asarray(bv, np.float32).reshape(1, C, 1)
    return out.reshape(B, C, H, W)


def run(x, Wq, bq, Wk, bk, Wv, bv, **kwargs):
    with_bias = bool(np.any(np.asarray(bq)) or np.any(np.asarray(bk)))
    key = f"nc{int(with_bias)}"
    nc = _CACHE.get(key)
    if nc is None:
        nc = build_program(with_bias=with_bias)
        _CACHE[key] = nc
    maps = _in_maps(x, Wq, bq, Wk, bk, Wv, bv)
    res = run_bass_kernel_spmd(nc, maps, core_ids=list(range(NCORES)), **kwargs)
    return _gather(res.results, bv), res


def kernel(x, Wq, bq, Wk, bk, Wv, bv) -> np.ndarray:
    out, _ = run(x, Wq, bq, Wk, bk, Wv, bv)
    return out


# revision 22
# speedup vs baseline: 1.1981x; 1.1981x over previous
"""Trainium2 Bass kernel for nn_AttentionModule (B=4, C=256, 64x64 spatial).

Reference computation (per batch b, x flattened to [C, HW]):
    q = Wq @ x + bq            [32, HW] -> per-pixel queries
    k = Wk @ x + bk            [32, HW]
    v = x^T @ Wv^T + bv        [HW, 256]
    out = softmax(q^T @ k) @ v [HW, 256] -> transposed to [C, HW]

Sharding: 8 cores, data-parallel over (batch, query-half): core = 2*b + h
computes queries [h*2048, (h+1)*2048) of batch b against all 4096 keys.
Weights replicated. The per-core q slice arrives as separate input data
(xq) so the program stays SPMD-identical.

Numerics: fp16 inputs/projections (5e-4 rounding), fp32 PSUM accumulate,
bf16 attention probabilities (fp16 would overflow: scores reach +-39).
Expected end-to-end ~5e-3 max-rel vs the fp32 reference.

Device layout:
  - scores computed transposed ([keys, q]) so the softmax denominator is
    accumulated by the PE itself: v carries ones columns, out[:, 256] =
    sum_k exp(s). exp on ScalarE straight out of PSUM, no max-subtraction
    (|s| <= ~40 is safe in fp32).
  - QK is 2-way row-packed: k tiles 0-15 live at partitions 0-31, tiles
    16-31 at partitions 32-63 (tile_position row groups), with q
    replicated to both blocks. Two K=32 matmuls run concurrently in the
    PE array; one [128, 1024] ACTIVATE converts both score tiles.
  - out tiles are [q, 258] in PSUM; normalization is per-partition
    reciprocal + tensor_scalar multiply on VectorE, fp32.
  - final [q, c] -> [c, q] transpose + bv bias happen host-side in the
    unshard step.
"""
import numpy as np
from contextlib import ExitStack

import concourse.bass as bass
import concourse.bacc as bacc
import concourse.tile as tile
from concourse import mybir
from concourse.bass_utils import run_bass_kernel_spmd

B, C, H, W = 4, 256, 64, 64
HW = H * W            # 4096
D = C // 8            # 32 (q/k channels)
NCORES = 8
Q = HW // 2           # 2048 queries per core
QC = 512              # q chunk (matmul moving dim)
NCH = Q // QC         # 4 chunks
KT = HW // 128        # 32 key tiles
P = 128
VW = C + 2            # v tile width (ones col + even-pad)

F32 = mybir.dt.float32
F16 = mybir.dt.float16
BF16 = mybir.dt.bfloat16
EXP = mybir.ActivationFunctionType.Exp

_CACHE: dict = {}


def build_program(with_bias: bool = False) -> bacc.Bacc:
    nc = bacc.Bacc("TRN2", target_bir_lowering=False, debug=False)

    xkv_d = nc.dram_tensor("xkv", [C, HW], F16, kind="ExternalInput").ap()
    xq_d = nc.dram_tensor("xq", [C, Q], F16, kind="ExternalInput").ap()
    # packed per c'-half: [wqT | wkT | wvT]  [256, 320]
    wpk_d = nc.dram_tensor("wpk", [C, 2 * D + C], F16, kind="ExternalInput").ap()
    # packed [bq | bk | ones(QC)]
    bpk_d = nc.dram_tensor("bpk", [1, 2 * D + QC], F16, kind="ExternalInput").ap()
    o_d = nc.dram_tensor("o", [Q, C], F16, kind="ExternalOutput").ap()

    with tile.TileContext(nc) as tc:
        with ExitStack() as ctx:
            big = ctx.enter_context(tc.tile_pool(name="big", bufs=24))
            const = ctx.enter_context(tc.tile_pool(name="const", bufs=1))
            ep = ctx.enter_context(tc.tile_pool(name="ep", bufs=4))
            ps = ctx.enter_context(tc.tile_pool(name="ps", bufs=2, space="PSUM"))
            po = ctx.enter_context(tc.tile_pool(name="po", bufs=4, space="PSUM"))

            # ---- PE warm-up: the HAM clock gate releases only under
            # sustained full-array activity; run dummy matmuls on a zeroed
            # tile while the input DMAs are still in flight ----
            dummy = const.tile([P, QC], F16, tag="dummy")
            nc.vector.memset(dummy[:], 0.0)
            wps = po.tile([P, QC], F32, tag="o", name="wps")
            for _ in range(8):
                nc.tensor.matmul(wps[:], dummy[:, 0:P], dummy[:],
                                 start=True, stop=True)

            # ---- constants / weights (3 triggers total) ----
            wpk_t = [const.tile([P, 2 * D + C], F16, tag=f"wpk{i}", name=f"wpk{i}")
                     for i in range(2)]
            for i in range(2):
                nc.scalar.dma_start(wpk_t[i][:], wpk_d[i * P:(i + 1) * P, :])
            bpk_t = const.tile([1, 2 * D + QC], F16, tag="bpk")
            nc.sync.dma_start(bpk_t[:], bpk_d)
            wq_sb = [wpk_t[i][:, 0:D] for i in range(2)]
            wk_sb = [wpk_t[i][:, D:2 * D] for i in range(2)]
            wv_sb = [wpk_t[i][:, 2 * D:] for i in range(2)]
            bq_sb = bpk_t[:, 0:D]
            bk_sb = bpk_t[:, D:2 * D]
            ones_sb = bpk_t[:, 2 * D:]

            # qrep: q^T replicated at partition blocks 0-31 and 32-63
            qrep = const.tile([2 * D, Q], F16, tag="qrep")
            # kT4: EVEN k tiles at partitions 0-31 (col (kt//2)*128),
            #      ODD  k tiles at partitions 32-63
            kT4 = const.tile([2 * D, 16 * P], F16, tag="kT4")
            kstage = const.tile([D, 16 * P], F16, tag="kstage")
            v_all = const.tile([P, KT * VW], F16, tag="vall")
            nc.vector.memset(
                v_all[:].rearrange("p (k c) -> p k c", c=VW)[:, :, C:C + 2], 1.0)

            # ---- x: two big const tiles per half, split across both HWDGE
            # queues (sync=SP, scalar=ACT; ACT is idle during the load) ----
            xkv_t = [const.tile([P, HW], F16, tag=f"xkv{i}", name=f"xkv{i}")
                     for i in range(2)]
            xq_t = [const.tile([P, Q], F16, tag=f"xq{i}", name=f"xq{i}")
                    for i in range(2)]
            eng = [nc.sync, nc.scalar]
            QT4 = HW // 4
            for quarter in (0, 1, 2, 3):
                for i in range(2):
                    eng[i].dma_start(
                        xkv_t[i][:, quarter * QT4:(quarter + 1) * QT4],
                        xkv_d[i * P:(i + 1) * P, quarter * QT4:(quarter + 1) * QT4])
                if quarter == 0:
                    for i in range(2):
                        eng[i].dma_start(xq_t[i][:, 0:QC], xq_d[i * P:(i + 1) * P, 0:QC])
            for i in range(2):
                eng[i].dma_start(xq_t[i][:, QC:], xq_d[i * P:(i + 1) * P, QC:])
            xq_sb = [[xq_t[i][:, j * QC:(j + 1) * QC] for j in range(Q // QC)]
                     for i in range(2)]
            xkv_sb = [[xkv_t[i][:, j * QC:(j + 1) * QC] for j in range(HW // QC)]
                      for i in range(2)]

            # ---- projections (PE, fp16 in / f32 psum) ----
            def kproj(j):
                kp = po.tile([D, QC], F32, tag="o", name="kp")
                nc.tensor.matmul(kp[:], wk_sb[0], xkv_sb[0][j],
                                 start=True, stop=not with_bias)
                nc.tensor.matmul(kp[:], wk_sb[1], xkv_sb[1][j],
                                 start=False, stop=not with_bias)
                if with_bias:
                    nc.tensor.matmul(kp[:], bk_sb, ones_sb, start=False, stop=True)
                # chunk j = k tiles 4j..4j+3: evens -> kT4[0:32], odds -> stage
                src = kp[:].rearrange("p (t c) -> p t c", c=P)
                dst_e = kT4[0:D, 2 * j * P:(2 * j + 2) * P].rearrange(
                    "p (t c) -> p t c", c=P)
                dst_o = kstage[:, 2 * j * P:(2 * j + 2) * P].rearrange(
                    "p (t c) -> p t c", c=P)
                nc.vector.tensor_copy(dst_o, src[:, 1::2])
                nc.vector.tensor_copy(dst_e, src[:, 0::2])
                nc.sync.dma_start(kT4[D:2 * D, 2 * j * P:(2 * j + 2) * P],
                                  kstage[:, 2 * j * P:(2 * j + 2) * P])

            def qproj(j):
                qp = po.tile([D, QC], F32, tag="o", name="qp")
                nc.tensor.matmul(qp[:], wq_sb[0], xq_sb[0][j],
                                 start=True, stop=not with_bias)
                nc.tensor.matmul(qp[:], wq_sb[1], xq_sb[1][j],
                                 start=False, stop=not with_bias)
                if with_bias:
                    nc.tensor.matmul(qp[:], bq_sb, ones_sb, start=False, stop=True)
                nc.vector.tensor_copy(qrep[0:D, j * QC:(j + 1) * QC], qp[:])
                nc.sync.dma_start(qrep[D:2 * D, j * QC:(j + 1) * QC],
                                  qrep[0:D, j * QC:(j + 1) * QC])

            v_sb = [v_all[:, t * VW:(t + 1) * VW] for t in range(KT)]

            def vproj(t):
                j, off = divmod(t, QC // P)
                vp = po.tile([P, C], F32, tag="o", name="vp")
                nc.tensor.matmul(
                    vp[:], xkv_sb[0][j][:, off * P:(off + 1) * P], wv_sb[0],
                    start=True, stop=False)
                nc.tensor.matmul(
                    vp[:], xkv_sb[1][j][:, off * P:(off + 1) * P], wv_sb[1],
                    start=False, stop=True)
                nc.vector.tensor_copy(v_sb[t][:, 0:C], vp[:])

            def vpair(g):
                vproj(2 * g)
                vproj(2 * g + 1)

            # chunk-0 deps first; the rest interleaves into the early
            # attention steps (all proj psum use ends before AV claims po).
            # With the even/odd pairing everything streams in natural order:
            # QK pair g needs kproj(g//2), AV pair g needs vpair(g).
            kproj(0)
            qproj(0)
            proj_work = [
                lambda: vpair(0), lambda: vpair(1),
                lambda: kproj(1), lambda: vpair(2),
                lambda: qproj(1), lambda: vpair(3),
                lambda: kproj(2), lambda: vpair(4), lambda: vpair(5),
                lambda: kproj(3), lambda: vpair(6),
                lambda: qproj(2), lambda: vpair(7),
                lambda: kproj(4), lambda: vpair(8), lambda: vpair(9),
                lambda: kproj(5), lambda: vpair(10),
                lambda: qproj(3), lambda: vpair(11),
                lambda: kproj(6), lambda: vpair(12), lambda: vpair(13),
                lambda: kproj(7), lambda: vpair(14), lambda: vpair(15),
            ]

            # ---- attention: flat 64-pair stream, AV lags QK by 2 pairs ----
            # pair (ci, g) = exp(scores) for k tiles (g, 16+g) of q chunk ci,
            # one [128, 1024] bf16 tile. AV of pair g covers kt=g and kt=16+g;
            # kt accumulation order [0,16,1,17,...] so start is kt==0 (pos 0)
            # and stop is kt==31 (pos 31).
            NP = NCH * 16
            AV_START = 18       # po banks stay proj-owned before this step

            def av_epi_half(ops2, ci, qlo):
                for k, op in enumerate(ops2):
                    qs = qlo + k
                    rinv = ep.tile([P, 1], F32, tag="rinv", name="rinv")
                    nc.vector.reciprocal(rinv[:], op[:, C:C + 1])
                    osb = ep.tile([P, C], F16, tag="osb", name="osb")
                    nc.vector.tensor_scalar_mul(osb[:], op[:, 0:C], rinv[:])
                    q0 = (ci * (QC // P) + qs) * P
                    nc.sync.dma_start(o_d[q0:q0 + P, :], osb[:])

            def av_half(ops2, pair_tile, g, qlo):
                for kt, half in ((2 * g, 0), (2 * g + 1, 1)):
                    for k in range(2):
                        qs = qlo + k
                        nc.tensor.matmul(
                            ops2[k][:],
                            pair_tile[:, half * QC + qs * P: half * QC + (qs + 1) * P],
                            v_sb[kt][:],
                            start=(kt == 0), stop=(kt == KT - 1))

            pair_tiles = {}
            ops1 = {}          # chunk -> sweep-1 accumulators (qs 0,1)
            ops2 = {}          # chunk -> sweep-2 accumulators (qs 2,3)
            s1 = 0             # pairs finished by sweep 1
            s2 = 0             # pairs finished by sweep 2
            step = 0
            wi = 0
            while s2 < NP:
                if step < NP:
                    ci, g = divmod(step, 16)
                    sc = ps.tile([P, 2 * QC], F32, tag="p", name="sc")
                    nc.tensor.matmul(
                        sc[:, 0:QC], kT4[0:D, g * P:(g + 1) * P],
                        qrep[0:D, ci * QC:(ci + 1) * QC],
                        start=True, stop=True, tile_position=(0, 0))
                    nc.tensor.matmul(
                        sc[:, QC:2 * QC], kT4[D:2 * D, g * P:(g + 1) * P],
                        qrep[D:2 * D, ci * QC:(ci + 1) * QC],
                        start=True, stop=True, tile_position=(D, 0))
                    Pt = big.tile([P, 2 * QC], BF16, tag="big", name="pt")
                    nc.scalar.activation(Pt[:], sc[:], EXP)
                    pair_tiles[step] = Pt
                for _ in range(2):
                    if wi < len(proj_work):
                        proj_work[wi]()
                        wi += 1
                # sweep 1: lag-2 behind QK, up to 2 pairs/step
                b = 2
                while b > 0 and s1 < NP and s1 <= step - 2:
                    c1, g1 = divmod(s1, 16)
                    if g1 == 0:
                        ops1[c1] = [po.tile([P, VW], F32, tag="o", name="av1")
                                    for _ in range(2)]
                    av_half(ops1[c1], pair_tiles[s1], g1, 0)
                    if g1 == 15:
                        av_epi_half(ops1.pop(c1), c1, 0)
                    s1 += 1
                    b -= 1
                # sweep 2: starts once proj has released the po slots,
                # then hugs sweep 1 at a 2-pair distance
                b = 2 if step >= 18 else 0
                s2_lim = s1 if s1 == NP else s1 - 2
                while b > 0 and s2 < s2_lim:
                    c2, g2 = divmod(s2, 16)
                    if g2 == 0:
                        ops2[c2] = [po.tile([P, VW], F32, tag="o", name="av2")
                                    for _ in range(2)]
                    av_half(ops2[c2], pair_tiles.pop(s2), g2, 2)
                    if g2 == 15:
                        av_epi_half(ops2.pop(c2), c2, 2)
                    s2 += 1
                    b -= 1
                step += 1

    nc.compile()
    return nc


def _in_maps(x, Wq, bq, Wk, bk, Wv, bv):
    xf = np.ascontiguousarray(np.asarray(x, np.float32).reshape(B, C, HW)).astype(np.float16)
    wpk = np.concatenate([
        np.asarray(Wq, np.float32).T,
        np.asarray(Wk, np.float32).T,
        np.asarray(Wv, np.float32).T], axis=1).astype(np.float16)
    bpk = np.concatenate([
        np.asarray(bq, np.float32).reshape(1, D),
        np.asarray(bk, np.float32).reshape(1, D),
        np.ones((1, QC), np.float32)], axis=1).astype(np.float16)
    maps = []
    for core in range(NCORES):
        b, h = divmod(core, 2)
        maps.append({
            "xkv": xf[b],
            "xq": np.ascontiguousarray(xf[b][:, h * Q:(h + 1) * Q]),
            "wpk": np.ascontiguousarray(wpk),
            "bpk": np.ascontiguousarray(bpk),
        })
    return maps


def _gather(results, bv):
    out = np.empty((B, C, HW), np.float32)
    for core in range(NCORES):
        b, h = divmod(core, 2)
        out[b][:, h * Q:(h + 1) * Q] = results[core]["o"].T
    out += np.asarray(bv, np.float32).reshape(1, C, 1)
    return out.reshape(B, C, H, W)


def run(x, Wq, bq, Wk, bk, Wv, bv, **kwargs):
    with_bias = bool(np.any(np.asarray(bq)) or np.any(np.asarray(bk)))
    key = f"nc{int(with_bias)}"
    nc = _CACHE.get(key)
    if nc is None:
        nc = build_program(with_bias=with_bias)
        _CACHE[key] = nc
    maps = _in_maps(x, Wq, bq, Wk, bk, Wv, bv)
    res = run_bass_kernel_spmd(nc, maps, core_ids=list(range(NCORES)), **kwargs)
    return _gather(res.results, bv), res


def kernel(x, Wq, bq, Wk, bk, Wv, bv) -> np.ndarray:
    out, _ = run(x, Wq, bq, Wk, bk, Wv, bv)
    return out


# revision 24
# speedup vs baseline: 1.2233x; 1.0210x over previous
"""Trainium2 Bass kernel for nn_AttentionModule (B=4, C=256, 64x64 spatial).

Reference computation (per batch b, x flattened to [C, HW]):
    q = Wq @ x + bq            [32, HW] -> per-pixel queries
    k = Wk @ x + bk            [32, HW]
    v = x^T @ Wv^T + bv        [HW, 256]
    out = softmax(q^T @ k) @ v [HW, 256] -> transposed to [C, HW]

Sharding: 8 cores, data-parallel over (batch, query-half): core = 2*b + h
computes queries [h*2048, (h+1)*2048) of batch b against all 4096 keys.
Weights replicated. The per-core q slice arrives as separate input data
(xq) so the program stays SPMD-identical.

Numerics: fp16 inputs/projections (5e-4 rounding), fp32 PSUM accumulate,
bf16 attention probabilities (fp16 would overflow: scores reach +-39).
Expected end-to-end ~5e-3 max-rel vs the fp32 reference.

Device layout:
  - scores computed transposed ([keys, q]) so the softmax denominator is
    accumulated by the PE itself: v carries ones columns, out[:, 256] =
    sum_k exp(s). exp on ScalarE straight out of PSUM, no max-subtraction
    (|s| <= ~40 is safe in fp32).
  - QK is 2-way row-packed: k tiles 0-15 live at partitions 0-31, tiles
    16-31 at partitions 32-63 (tile_position row groups), with q
    replicated to both blocks. Two K=32 matmuls run concurrently in the
    PE array; one [128, 1024] ACTIVATE converts both score tiles.
  - out tiles are [q, 258] in PSUM; normalization is per-partition
    reciprocal + tensor_scalar multiply on VectorE, fp32.
  - final [q, c] -> [c, q] transpose + bv bias happen host-side in the
    unshard step.
"""
import numpy as np
from contextlib import ExitStack

import concourse.bass as bass
import concourse.bacc as bacc
import concourse.tile as tile
from concourse import mybir
from concourse.bass_utils import run_bass_kernel_spmd

B, C, H, W = 4, 256, 64, 64
HW = H * W            # 4096
D = C // 8            # 32 (q/k channels)
NCORES = 8
Q = HW // 2           # 2048 queries per core
QC = 512              # q chunk (matmul moving dim)
NCH = Q // QC         # 4 chunks
KT = HW // 128        # 32 key tiles
P = 128
VW = C + 2            # v tile width (ones col + even-pad)

F32 = mybir.dt.float32
F16 = mybir.dt.float16
BF16 = mybir.dt.bfloat16
EXP = mybir.ActivationFunctionType.Exp

_CACHE: dict = {}


def build_program(with_bias: bool = False) -> bacc.Bacc:
    nc = bacc.Bacc("TRN2", target_bir_lowering=False, debug=False)

    xkv_d = nc.dram_tensor("xkv", [C, HW], F16, kind="ExternalInput").ap()
    xq_d = nc.dram_tensor("xq", [C, Q], F16, kind="ExternalInput").ap()
    # packed per c'-half: [wqT | wkT | wvT]  [256, 320]
    wpk_d = nc.dram_tensor("wpk", [C, 2 * D + C], F16, kind="ExternalInput").ap()
    # packed [bq | bk | ones(QC)]
    bpk_d = nc.dram_tensor("bpk", [1, 2 * D + QC], F16, kind="ExternalInput").ap()
    o_d = nc.dram_tensor("o", [Q, C], F16, kind="ExternalOutput").ap()

    with tile.TileContext(nc) as tc:
        with ExitStack() as ctx:
            big = ctx.enter_context(tc.tile_pool(name="big", bufs=24))
            const = ctx.enter_context(tc.tile_pool(name="const", bufs=1))
            ep = ctx.enter_context(tc.tile_pool(name="ep", bufs=4))
            ps = ctx.enter_context(tc.tile_pool(name="ps", bufs=2, space="PSUM"))
            po = ctx.enter_context(tc.tile_pool(name="po", bufs=4, space="PSUM"))

            # ---- PE warm-up: the HAM clock gate releases only under
            # sustained full-array activity; run dummy matmuls on a zeroed
            # tile while the input DMAs are still in flight ----
            dummy = const.tile([P, QC], F16, tag="dummy")
            nc.vector.memset(dummy[:], 0.0)
            wps = po.tile([P, QC], F32, tag="o", name="wps")
            for _ in range(8):
                nc.tensor.matmul(wps[:], dummy[:, 0:P], dummy[:],
                                 start=True, stop=True)

            # ---- constants / weights (3 triggers total) ----
            wpk_t = [const.tile([P, 2 * D + C], F16, tag=f"wpk{i}", name=f"wpk{i}")
                     for i in range(2)]
            for i in range(2):
                nc.scalar.dma_start(wpk_t[i][:], wpk_d[i * P:(i + 1) * P, :])
            bpk_t = const.tile([1, 2 * D + QC], F16, tag="bpk")
            nc.sync.dma_start(bpk_t[:], bpk_d)
            wq_sb = [wpk_t[i][:, 0:D] for i in range(2)]
            wk_sb = [wpk_t[i][:, D:2 * D] for i in range(2)]
            wv_sb = [wpk_t[i][:, 2 * D:] for i in range(2)]
            bq_sb = bpk_t[:, 0:D]
            bk_sb = bpk_t[:, D:2 * D]
            ones_sb = bpk_t[:, 2 * D:]

            # qrep: q^T replicated at partition blocks 0-31 and 32-63
            qrep = const.tile([2 * D, Q], F16, tag="qrep")
            # kT4: EVEN k tiles at partitions 0-31 (col (kt//2)*128),
            #      ODD  k tiles at partitions 32-63
            kT4 = const.tile([2 * D, 16 * P], F16, tag="kT4")
            kstage = const.tile([D, 16 * P], F16, tag="kstage")
            v_all = const.tile([P, KT * VW], F16, tag="vall")
            nc.vector.memset(
                v_all[:].rearrange("p (k c) -> p k c", c=VW)[:, :, C:C + 2], 1.0)

            # ---- x: two big const tiles per half, split across both HWDGE
            # queues (sync=SP, scalar=ACT; ACT is idle during the load) ----
            xkv_t = [const.tile([P, HW], F16, tag=f"xkv{i}", name=f"xkv{i}")
                     for i in range(2)]
            xq_t = [const.tile([P, Q], F16, tag=f"xq{i}", name=f"xq{i}")
                    for i in range(2)]
            eng = [nc.sync, nc.scalar]
            QT4 = HW // 4
            for quarter in (0, 1, 2, 3):
                for i in range(2):
                    eng[i].dma_start(
                        xkv_t[i][:, quarter * QT4:(quarter + 1) * QT4],
                        xkv_d[i * P:(i + 1) * P, quarter * QT4:(quarter + 1) * QT4])
                if quarter == 0:
                    for i in range(2):
                        eng[i].dma_start(xq_t[i][:, 0:QC], xq_d[i * P:(i + 1) * P, 0:QC])
            for i in range(2):
                eng[i].dma_start(xq_t[i][:, QC:], xq_d[i * P:(i + 1) * P, QC:])
            xq_sb = [[xq_t[i][:, j * QC:(j + 1) * QC] for j in range(Q // QC)]
                     for i in range(2)]
            xkv_sb = [[xkv_t[i][:, j * QC:(j + 1) * QC] for j in range(HW // QC)]
                      for i in range(2)]

            # ---- projections (PE, fp16 in / f32 psum) ----
            def kproj(j):
                kp = po.tile([D, QC], F32, tag="o", name="kp")
                nc.tensor.matmul(kp[:], wk_sb[0], xkv_sb[0][j],
                                 start=True, stop=not with_bias)
                nc.tensor.matmul(kp[:], wk_sb[1], xkv_sb[1][j],
                                 start=False, stop=not with_bias)
                if with_bias:
                    nc.tensor.matmul(kp[:], bk_sb, ones_sb, start=False, stop=True)
                # chunk j = k tiles 4j..4j+3: evens -> kT4[0:32], odds -> stage
                src = kp[:].rearrange("p (t c) -> p t c", c=P)
                dst_e = kT4[0:D, 2 * j * P:(2 * j + 2) * P].rearrange(
                    "p (t c) -> p t c", c=P)
                dst_o = kstage[:, 2 * j * P:(2 * j + 2) * P].rearrange(
                    "p (t c) -> p t c", c=P)
                nc.vector.tensor_copy(dst_o, src[:, 1::2])
                nc.vector.tensor_copy(dst_e, src[:, 0::2])
                nc.sync.dma_start(kT4[D:2 * D, 2 * j * P:(2 * j + 2) * P],
                                  kstage[:, 2 * j * P:(2 * j + 2) * P])

            def qproj(j):
                qp = po.tile([D, QC], F32, tag="o", name="qp")
                nc.tensor.matmul(qp[:], wq_sb[0], xq_sb[0][j],
                                 start=True, stop=not with_bias)
                nc.tensor.matmul(qp[:], wq_sb[1], xq_sb[1][j],
                                 start=False, stop=not with_bias)
                if with_bias:
                    nc.tensor.matmul(qp[:], bq_sb, ones_sb, start=False, stop=True)
                nc.vector.tensor_copy(qrep[0:D, j * QC:(j + 1) * QC], qp[:])
                nc.sync.dma_start(qrep[D:2 * D, j * QC:(j + 1) * QC],
                                  qrep[0:D, j * QC:(j + 1) * QC])

            v_sb = [v_all[:, t * VW:(t + 1) * VW] for t in range(KT)]

            def vproj(t):
                j, off = divmod(t, QC // P)
                vp = po.tile([P, C], F32, tag="o", name="vp")
                nc.tensor.matmul(
                    vp[:], xkv_sb[0][j][:, off * P:(off + 1) * P], wv_sb[0],
                    start=True, stop=False)
                nc.tensor.matmul(
                    vp[:], xkv_sb[1][j][:, off * P:(off + 1) * P], wv_sb[1],
                    start=False, stop=True)
                nc.vector.tensor_copy(v_sb[t][:, 0:C], vp[:])

            def vpair(g):
                vproj(2 * g)
                vproj(2 * g + 1)

            # chunk-0 deps first; the rest interleaves into the early
            # attention steps (all proj psum use ends before AV claims po).
            # With the even/odd pairing everything streams in natural order:
            # QK pair g needs kproj(g//2), AV pair g needs vpair(g).
            kproj(0)
            qproj(0)
            proj_work = [
                lambda: vpair(0), lambda: vpair(1),
                lambda: kproj(1), lambda: vpair(2),
                lambda: qproj(1), lambda: vpair(3),
                lambda: kproj(2), lambda: vpair(4), lambda: vpair(5),
                lambda: kproj(3), lambda: vpair(6),
                lambda: qproj(2), lambda: vpair(7),
                lambda: kproj(4), lambda: vpair(8), lambda: vpair(9),
                lambda: kproj(5), lambda: vpair(10),
                lambda: qproj(3), lambda: vpair(11),
                lambda: kproj(6), lambda: vpair(12), lambda: vpair(13),
                lambda: kproj(7), lambda: vpair(14), lambda: vpair(15),
            ]

            # ---- attention: flat 64-pair stream, AV lags QK by 2 pairs ----
            # pair (ci, g) = exp(scores) for k tiles (g, 16+g) of q chunk ci,
            # one [128, 1024] bf16 tile. AV of pair g covers kt=g and kt=16+g;
            # kt accumulation order [0,16,1,17,...] so start is kt==0 (pos 0)
            # and stop is kt==31 (pos 31).
            NP = NCH * 16
            AV_START = 18       # po banks stay proj-owned before this step

            def av_epilogue(ops, ci):
                for qs in range(QC // P):
                    op = ops[qs]
                    rinv = ep.tile([P, 1], F32, tag="rinv", name="rinv")
                    nc.vector.reciprocal(rinv[:], op[:, C:C + 1])
                    osb = ep.tile([P, C], F16, tag="osb", name="osb")
                    nc.vector.tensor_scalar_mul(osb[:], op[:, 0:C], rinv[:])
                    q0 = (ci * (QC // P) + qs) * P
                    nc.sync.dma_start(o_d[q0:q0 + P, :], osb[:])

            def av_pair(ops, pair_tile, g):
                for kt, half in ((2 * g, 0), (2 * g + 1, 1)):
                    for qs in range(QC // P):
                        nc.tensor.matmul(
                            ops[qs][:],
                            pair_tile[:, half * QC + qs * P: half * QC + (qs + 1) * P],
                            v_sb[kt][:],
                            start=(kt == 0), stop=(kt == KT - 1))

            pair_tiles = {}
            ops = {}
            av_done = 0
            step = 0
            wi = 0
            while av_done < NP:
                if step < NP:
                    ci, g = divmod(step, 16)
                    sc = ps.tile([P, 2 * QC], F32, tag="p", name="sc")
                    nc.tensor.matmul(
                        sc[:, 0:QC], kT4[0:D, g * P:(g + 1) * P],
                        qrep[0:D, ci * QC:(ci + 1) * QC],
                        start=True, stop=True, tile_position=(0, 0))
                    nc.tensor.matmul(
                        sc[:, QC:2 * QC], kT4[D:2 * D, g * P:(g + 1) * P],
                        qrep[D:2 * D, ci * QC:(ci + 1) * QC],
                        start=True, stop=True, tile_position=(D, 0))
                    Pt = big.tile([P, 2 * QC], BF16, tag="big", name="pt")
                    nc.scalar.activation(Pt[:], sc[:], EXP)
                    pair_tiles[step] = Pt
                for _ in range(2):
                    if wi < len(proj_work):
                        proj_work[wi]()
                        wi += 1
                budget = 2 if step >= AV_START else 0
                while budget > 0 and av_done < NP and av_done <= step - 2:
                    cav, gav = divmod(av_done, 16)
                    if gav == 0:
                        ops[cav] = [po.tile([P, VW], F32, tag="o", name="avo")
                                    for _ in range(QC // P)]
                    av_pair(ops[cav], pair_tiles.pop(av_done), gav)
                    if gav == 15:
                        av_epilogue(ops.pop(cav), cav)
                    av_done += 1
                    budget -= 1
                step += 1

    nc.compile()
    return nc


def _in_maps(x, Wq, bq, Wk, bk, Wv, bv):
    xf = np.ascontiguousarray(np.asarray(x, np.float32).reshape(B, C, HW)).astype(np.float16)
    wpk = np.concatenate([
        np.asarray(Wq, np.float32).T,
        np.asarray(Wk, np.float32).T,
        np.asarray(Wv, np.float32).T], axis=1).astype(np.float16)
    bpk = np.concatenate([
        np.asarray(bq, np.float32).reshape(1, D),
        np.asarray(bk, np.float32).reshape(1, D),
        np.ones((1, QC), np.float32)], axis=1).astype(np.float16)
    maps = []
    for core in range(NCORES):
        b, h = divmod(core, 2)
        maps.append({
            "xkv": xf[b],
            "xq": np.ascontiguousarray(xf[b][:, h * Q:(h + 1) * Q]),
            "wpk": np.ascontiguousarray(wpk),
            "bpk": np.ascontiguousarray(bpk),
        })
    return maps


def _gather(results, bv):
    out = np.empty((B, C, HW), np.float32)
    for core in range(NCORES):
        b, h = divmod(core, 2)
        out[b][:, h * Q:(h + 1) * Q] = results[core]["o"].T
    out += np.asarray(bv, np.float32).reshape(1, C, 1)
    return out.reshape(B, C, H, W)


def run(x, Wq, bq, Wk, bk, Wv, bv, **kwargs):
    with_bias = bool(np.any(np.asarray(bq)) or np.any(np.asarray(bk)))
    key = f"nc{int(with_bias)}"
    nc = _CACHE.get(key)
    if nc is None:
        nc = build_program(with_bias=with_bias)
        _CACHE[key] = nc
    maps = _in_maps(x, Wq, bq, Wk, bk, Wv, bv)
    res = run_bass_kernel_spmd(nc, maps, core_ids=list(range(NCORES)), **kwargs)
    return _gather(res.results, bv), res


def kernel(x, Wq, bq, Wk, bk, Wv, bv) -> np.ndarray:
    out, _ = run(x, Wq, bq, Wk, bk, Wv, bv)
    return out


# revision 25
# speedup vs baseline: 1.2349x; 1.0094x over previous
"""Trainium2 Bass kernel for nn_AttentionModule (B=4, C=256, 64x64 spatial).

Reference computation (per batch b, x flattened to [C, HW]):
    q = Wq @ x + bq            [32, HW] -> per-pixel queries
    k = Wk @ x + bk            [32, HW]
    v = x^T @ Wv^T + bv        [HW, 256]
    out = softmax(q^T @ k) @ v [HW, 256] -> transposed to [C, HW]

Sharding: 8 cores, data-parallel over (batch, query-half): core = 2*b + h
computes queries [h*2048, (h+1)*2048) of batch b against all 4096 keys.
Weights replicated. The per-core q slice arrives as separate input data
(xq) so the program stays SPMD-identical.

Numerics: fp16 inputs/projections (5e-4 rounding), fp32 PSUM accumulate,
bf16 attention probabilities (fp16 would overflow: scores reach +-39).
Expected end-to-end ~5e-3 max-rel vs the fp32 reference.

Device layout:
  - scores computed transposed ([keys, q]) so the softmax denominator is
    accumulated by the PE itself: v carries ones columns, out[:, 256] =
    sum_k exp(s). exp on ScalarE straight out of PSUM, no max-subtraction
    (|s| <= ~40 is safe in fp32).
  - QK is 2-way row-packed: k tiles 0-15 live at partitions 0-31, tiles
    16-31 at partitions 32-63 (tile_position row groups), with q
    replicated to both blocks. Two K=32 matmuls run concurrently in the
    PE array; one [128, 1024] ACTIVATE converts both score tiles.
  - out tiles are [q, 258] in PSUM; normalization is per-partition
    reciprocal + tensor_scalar multiply on VectorE, fp32.
  - final [q, c] -> [c, q] transpose + bv bias happen host-side in the
    unshard step.
"""
import numpy as np
from contextlib import ExitStack

import concourse.bass as bass
import concourse.bacc as bacc
import concourse.tile as tile
from concourse import mybir
from concourse.bass_utils import run_bass_kernel_spmd

B, C, H, W = 4, 256, 64, 64
HW = H * W            # 4096
D = C // 8            # 32 (q/k channels)
NCORES = 8
Q = HW // 2           # 2048 queries per core
QC = 512              # q chunk (matmul moving dim)
NCH = Q // QC         # 4 chunks
KT = HW // 128        # 32 key tiles
P = 128
VW = C + 2            # v tile width (ones col + even-pad)

F32 = mybir.dt.float32
F16 = mybir.dt.float16
BF16 = mybir.dt.bfloat16
EXP = mybir.ActivationFunctionType.Exp

_CACHE: dict = {}


def build_program(with_bias: bool = False) -> bacc.Bacc:
    nc = bacc.Bacc("TRN2", target_bir_lowering=False, debug=False)

    xkv_d = nc.dram_tensor("xkv", [C, HW], F16, kind="ExternalInput").ap()
    xq_d = nc.dram_tensor("xq", [C, Q], F16, kind="ExternalInput").ap()
    # packed per c'-half: [wqT | wkT | wvT]  [256, 320]
    wpk_d = nc.dram_tensor("wpk", [C, 2 * D + C], F16, kind="ExternalInput").ap()
    # packed [bq | bk | ones(QC)]
    bpk_d = nc.dram_tensor("bpk", [1, 2 * D + QC], F16, kind="ExternalInput").ap()
    o_d = nc.dram_tensor("o", [Q, C], F16, kind="ExternalOutput").ap()

    with tile.TileContext(nc) as tc:
        with ExitStack() as ctx:
            big = ctx.enter_context(tc.tile_pool(name="big", bufs=24))
            const = ctx.enter_context(tc.tile_pool(name="const", bufs=1))
            ep = ctx.enter_context(tc.tile_pool(name="ep", bufs=4))
            ps = ctx.enter_context(tc.tile_pool(name="ps", bufs=2, space="PSUM"))
            po = ctx.enter_context(tc.tile_pool(name="po", bufs=4, space="PSUM"))

            # ---- PE warm-up: the HAM clock gate releases only under
            # sustained full-array activity; run dummy matmuls on a zeroed
            # tile while the input DMAs are still in flight ----
            dummy = const.tile([P, QC], F16, tag="dummy")
            nc.vector.memset(dummy[:], 0.0)
            wps = po.tile([P, QC], F32, tag="o", name="wps")
            for _ in range(8):
                nc.tensor.matmul(wps[:], dummy[:, 0:P], dummy[:],
                                 start=True, stop=True)

            # ---- constants / weights (3 triggers total) ----
            wpk_t = [const.tile([P, 2 * D + C], F16, tag=f"wpk{i}", name=f"wpk{i}")
                     for i in range(2)]
            for i in range(2):
                nc.scalar.dma_start(wpk_t[i][:], wpk_d[i * P:(i + 1) * P, :])
            bpk_t = const.tile([1, 2 * D + QC], F16, tag="bpk")
            nc.sync.dma_start(bpk_t[:], bpk_d)
            wq_sb = [wpk_t[i][:, 0:D] for i in range(2)]
            wk_sb = [wpk_t[i][:, D:2 * D] for i in range(2)]
            wv_sb = [wpk_t[i][:, 2 * D:] for i in range(2)]
            bq_sb = bpk_t[:, 0:D]
            bk_sb = bpk_t[:, D:2 * D]
            ones_sb = bpk_t[:, 2 * D:]

            # qrep: q^T replicated at partition blocks 0-31 and 32-63
            qrep = const.tile([2 * D, Q], F16, tag="qrep")
            # kT4: EVEN k tiles at partitions 0-31 (col (kt//2)*128),
            #      ODD  k tiles at partitions 32-63
            kT4 = const.tile([2 * D, 16 * P], F16, tag="kT4")
            kstage = const.tile([D, 16 * P], F16, tag="kstage")
            v_all = const.tile([P, KT * VW], F16, tag="vall")
            nc.vector.memset(
                v_all[:].rearrange("p (k c) -> p k c", c=VW)[:, :, C:C + 2], 1.0)

            # ---- x: two big const tiles per half, split across both HWDGE
            # queues (sync=SP, scalar=ACT; ACT is idle during the load) ----
            xkv_t = [const.tile([P, HW], F16, tag=f"xkv{i}", name=f"xkv{i}")
                     for i in range(2)]
            xq_t = [const.tile([P, Q], F16, tag=f"xq{i}", name=f"xq{i}")
                    for i in range(2)]
            QT4 = HW // 4
            # Queue layout: scalar (ACT) gets only the minimal chunk-0
            # pieces so exps can start early; sync carries the rest IN
            # CONSUMPTION ORDER, with quarters 2-3 deferred into proj_work
            # so the kT4/qrep staging DMAs are not stuck behind them.
            # All deferred triggers go on sync - never scalar (an x trigger
            # behind an exp in the ACT stream deadlocks: exp waits QK waits
            # x-data waits trigger waits exp).
            nc.scalar.dma_start(xkv_t[1][:, 0:QT4], xkv_d[P:2 * P, 0:QT4])
            nc.scalar.dma_start(xq_t[1][:, 0:QC], xq_d[P:2 * P, 0:QC])
            nc.sync.dma_start(xkv_t[0][:, 0:QT4], xkv_d[0:P, 0:QT4])
            nc.sync.dma_start(xq_t[0][:, 0:QC], xq_d[0:P, 0:QC])
            for i in range(2):
                nc.sync.dma_start(xkv_t[i][:, QT4:2 * QT4],
                                  xkv_d[i * P:(i + 1) * P, QT4:2 * QT4])
                nc.sync.dma_start(xq_t[i][:, QC:], xq_d[i * P:(i + 1) * P, QC:])

            def xdma(quarter):
                for i in range(2):
                    nc.sync.dma_start(
                        xkv_t[i][:, quarter * QT4:(quarter + 1) * QT4],
                        xkv_d[i * P:(i + 1) * P, quarter * QT4:(quarter + 1) * QT4])
            xq_sb = [[xq_t[i][:, j * QC:(j + 1) * QC] for j in range(Q // QC)]
                     for i in range(2)]
            xkv_sb = [[xkv_t[i][:, j * QC:(j + 1) * QC] for j in range(HW // QC)]
                      for i in range(2)]

            # ---- projections (PE, fp16 in / f32 psum) ----
            def kproj(j):
                kp = po.tile([D, QC], F32, tag="o", name="kp")
                nc.tensor.matmul(kp[:], wk_sb[0], xkv_sb[0][j],
                                 start=True, stop=not with_bias)
                nc.tensor.matmul(kp[:], wk_sb[1], xkv_sb[1][j],
                                 start=False, stop=not with_bias)
                if with_bias:
                    nc.tensor.matmul(kp[:], bk_sb, ones_sb, start=False, stop=True)
                # chunk j = k tiles 4j..4j+3: evens -> kT4[0:32], odds -> stage
                src = kp[:].rearrange("p (t c) -> p t c", c=P)
                dst_e = kT4[0:D, 2 * j * P:(2 * j + 2) * P].rearrange(
                    "p (t c) -> p t c", c=P)
                dst_o = kstage[:, 2 * j * P:(2 * j + 2) * P].rearrange(
                    "p (t c) -> p t c", c=P)
                nc.vector.tensor_copy(dst_o, src[:, 1::2])
                nc.vector.tensor_copy(dst_e, src[:, 0::2])
                nc.sync.dma_start(kT4[D:2 * D, 2 * j * P:(2 * j + 2) * P],
                                  kstage[:, 2 * j * P:(2 * j + 2) * P])

            def qproj(j):
                qp = po.tile([D, QC], F32, tag="o", name="qp")
                nc.tensor.matmul(qp[:], wq_sb[0], xq_sb[0][j],
                                 start=True, stop=not with_bias)
                nc.tensor.matmul(qp[:], wq_sb[1], xq_sb[1][j],
                                 start=False, stop=not with_bias)
                if with_bias:
                    nc.tensor.matmul(qp[:], bq_sb, ones_sb, start=False, stop=True)
                nc.vector.tensor_copy(qrep[0:D, j * QC:(j + 1) * QC], qp[:])
                nc.sync.dma_start(qrep[D:2 * D, j * QC:(j + 1) * QC],
                                  qrep[0:D, j * QC:(j + 1) * QC])

            v_sb = [v_all[:, t * VW:(t + 1) * VW] for t in range(KT)]

            def vproj(t):
                j, off = divmod(t, QC // P)
                vp = po.tile([P, C], F32, tag="o", name="vp")
                nc.tensor.matmul(
                    vp[:], xkv_sb[0][j][:, off * P:(off + 1) * P], wv_sb[0],
                    start=True, stop=False)
                nc.tensor.matmul(
                    vp[:], xkv_sb[1][j][:, off * P:(off + 1) * P], wv_sb[1],
                    start=False, stop=True)
                nc.vector.tensor_copy(v_sb[t][:, 0:C], vp[:])

            def vpair(g):
                vproj(2 * g)
                vproj(2 * g + 1)

            # chunk-0 deps first; the rest interleaves into the early
            # attention steps (all proj psum use ends before AV claims po).
            # With the even/odd pairing everything streams in natural order:
            # QK pair g needs kproj(g//2), AV pair g needs vpair(g).
            kproj(0)
            qproj(0)
            proj_work = [
                lambda: xdma(2), lambda: vpair(0), lambda: vpair(1),
                lambda: kproj(1), lambda: vpair(2),
                lambda: qproj(1), lambda: vpair(3),
                lambda: kproj(2), lambda: vpair(4), lambda: vpair(5),
                lambda: xdma(3),
                lambda: kproj(3), lambda: vpair(6),
                lambda: qproj(2), lambda: vpair(7),
                lambda: kproj(4), lambda: vpair(8), lambda: vpair(9),
                lambda: kproj(5), lambda: vpair(10),
                lambda: qproj(3), lambda: vpair(11),
                lambda: kproj(6), lambda: vpair(12), lambda: vpair(13),
                lambda: kproj(7), lambda: vpair(14), lambda: vpair(15),
            ]

            # ---- attention: flat 64-pair stream, AV lags QK by 2 pairs ----
            # pair (ci, g) = exp(scores) for k tiles (g, 16+g) of q chunk ci,
            # one [128, 1024] bf16 tile. AV of pair g covers kt=g and kt=16+g;
            # kt accumulation order [0,16,1,17,...] so start is kt==0 (pos 0)
            # and stop is kt==31 (pos 31).
            NP = NCH * 16
            AV_START = 18       # po banks stay proj-owned before this step

            def av_epilogue(ops, ci):
                for qs in range(QC // P):
                    op = ops[qs]
                    rinv = ep.tile([P, 1], F32, tag="rinv", name="rinv")
                    nc.vector.reciprocal(rinv[:], op[:, C:C + 1])
                    osb = ep.tile([P, C], F16, tag="osb", name="osb")
                    nc.vector.tensor_scalar_mul(osb[:], op[:, 0:C], rinv[:])
                    q0 = (ci * (QC // P) + qs) * P
                    nc.sync.dma_start(o_d[q0:q0 + P, :], osb[:])

            def av_pair(ops, pair_tile, g):
                for kt, half in ((2 * g, 0), (2 * g + 1, 1)):
                    for qs in range(QC // P):
                        nc.tensor.matmul(
                            ops[qs][:],
                            pair_tile[:, half * QC + qs * P: half * QC + (qs + 1) * P],
                            v_sb[kt][:],
                            start=(kt == 0), stop=(kt == KT - 1))

            pair_tiles = {}
            ops = {}
            av_done = 0
            step = 0
            wi = 0
            while av_done < NP:
                if step < NP:
                    ci, g = divmod(step, 16)
                    sc = ps.tile([P, 2 * QC], F32, tag="p", name="sc")
                    nc.tensor.matmul(
                        sc[:, 0:QC], kT4[0:D, g * P:(g + 1) * P],
                        qrep[0:D, ci * QC:(ci + 1) * QC],
                        start=True, stop=True, tile_position=(0, 0))
                    nc.tensor.matmul(
                        sc[:, QC:2 * QC], kT4[D:2 * D, g * P:(g + 1) * P],
                        qrep[D:2 * D, ci * QC:(ci + 1) * QC],
                        start=True, stop=True, tile_position=(D, 0))
                    Pt = big.tile([P, 2 * QC], BF16, tag="big", name="pt")
                    nc.scalar.activation(Pt[:], sc[:], EXP)
                    pair_tiles[step] = Pt
                for _ in range(2):
                    if wi < len(proj_work):
                        proj_work[wi]()
                        wi += 1
                budget = 2 if step >= AV_START else 0
                while budget > 0 and av_done < NP and av_done <= step - 2:
                    cav, gav = divmod(av_done, 16)
                    if gav == 0:
                        ops[cav] = [po.tile([P, VW], F32, tag="o", name="avo")
                                    for _ in range(QC // P)]
                    av_pair(ops[cav], pair_tiles.pop(av_done), gav)
                    if gav == 15:
                        av_epilogue(ops.pop(cav), cav)
                    av_done += 1
                    budget -= 1
                step += 1

    nc.compile()
    return nc


def _in_maps(x, Wq, bq, Wk, bk, Wv, bv):
    xf = np.ascontiguousarray(np.asarray(x, np.float32).reshape(B, C, HW)).astype(np.float16)
    wpk = np.concatenate([
        np.asarray(Wq, np.float32).T,
        np.asarray(Wk, np.float32).T,
        np.asarray(Wv, np.float32).T], axis=1).astype(np.float16)
    bpk = np.concatenate([
        np.asarray(bq, np.float32).reshape(1, D),
        np.asarray(bk, np.float32).reshape(1, D),
        np.ones((1, QC), np.float32)], axis=1).astype(np.float16)
    maps = []
    for core in range(NCORES):
        b, h = divmod(core, 2)
        maps.append({
            "xkv": xf[b],
            "xq": np.ascontiguousarray(xf[b][:, h * Q:(h + 1) * Q]),
            "wpk": np.ascontiguousarray(wpk),
            "bpk": np.ascontiguousarray(bpk),
        })
    return maps


def _gather(results, bv):
    out = np.empty((B, C, HW), np.float32)
    for core in range(NCORES):
        b, h = divmod(core, 2)
        out[b][:, h * Q:(h + 1) * Q] = results[core]["o"].T
    out += np.asarray(bv, np.float32).reshape(1, C, 1)
    return out.reshape(B, C, H, W)


def run(x, Wq, bq, Wk, bk, Wv, bv, **kwargs):
    with_bias = bool(np.any(np.asarray(bq)) or np.any(np.asarray(bk)))
    key = f"nc{int(with_bias)}"
    nc = _CACHE.get(key)
    if nc is None:
        nc = build_program(with_bias=with_bias)
        _CACHE[key] = nc
    maps = _in_maps(x, Wq, bq, Wk, bk, Wv, bv)
    res = run_bass_kernel_spmd(nc, maps, core_ids=list(range(NCORES)), **kwargs)
    return _gather(res.results, bv), res


def kernel(x, Wq, bq, Wk, bk, Wv, bv) -> np.ndarray:
    out, _ = run(x, Wq, bq, Wk, bk, Wv, bv)
    return out


# revision 26
# speedup vs baseline: 1.2682x; 1.0270x over previous
"""Trainium2 Bass kernel for nn_AttentionModule (B=4, C=256, 64x64 spatial).

Reference computation (per batch b, x flattened to [C, HW]):
    q = Wq @ x + bq            [32, HW] -> per-pixel queries
    k = Wk @ x + bk            [32, HW]
    v = x^T @ Wv^T + bv        [HW, 256]
    out = softmax(q^T @ k) @ v [HW, 256] -> transposed to [C, HW]

Sharding: 8 cores, data-parallel over (batch, query-half): core = 2*b + h
computes queries [h*2048, (h+1)*2048) of batch b against all 4096 keys.
Weights replicated. The per-core q slice arrives as separate input data
(xq) so the program stays SPMD-identical.

Numerics: fp16 inputs/projections (5e-4 rounding), fp32 PSUM accumulate,
bf16 attention probabilities (fp16 would overflow: scores reach +-39).
Expected end-to-end ~5e-3 max-rel vs the fp32 reference.

Device layout:
  - scores computed transposed ([keys, q]) so the softmax denominator is
    accumulated by the PE itself: v carries ones columns, out[:, 256] =
    sum_k exp(s). exp on ScalarE straight out of PSUM, no max-subtraction
    (|s| <= ~40 is safe in fp32).
  - QK is 2-way row-packed: k tiles 0-15 live at partitions 0-31, tiles
    16-31 at partitions 32-63 (tile_position row groups), with q
    replicated to both blocks. Two K=32 matmuls run concurrently in the
    PE array; one [128, 1024] ACTIVATE converts both score tiles.
  - out tiles are [q, 258] in PSUM; normalization is per-partition
    reciprocal + tensor_scalar multiply on VectorE, fp32.
  - final [q, c] -> [c, q] transpose + bv bias happen host-side in the
    unshard step.
"""
import numpy as np
from contextlib import ExitStack

import concourse.bass as bass
import concourse.bacc as bacc
import concourse.tile as tile
from concourse import mybir
from concourse.bass_utils import run_bass_kernel_spmd

B, C, H, W = 4, 256, 64, 64
HW = H * W            # 4096
D = C // 8            # 32 (q/k channels)
NCORES = 8
Q = HW // 2           # 2048 queries per core
QC = 512              # q chunk (matmul moving dim)
NCH = Q // QC         # 4 chunks
KT = HW // 128        # 32 key tiles
P = 128
VW = C + 2            # v tile width (ones col + even-pad)

F32 = mybir.dt.float32
F16 = mybir.dt.float16
BF16 = mybir.dt.bfloat16
EXP = mybir.ActivationFunctionType.Exp

_CACHE: dict = {}


def build_program(with_bias: bool = False) -> bacc.Bacc:
    nc = bacc.Bacc("TRN2", target_bir_lowering=False, debug=False)

    xkv_d = nc.dram_tensor("xkv", [C, HW], F16, kind="ExternalInput").ap()
    xq_d = nc.dram_tensor("xq", [C, Q], F16, kind="ExternalInput").ap()
    # packed per c'-half: [wqT | wkT | wvT]  [256, 320]
    wpk_d = nc.dram_tensor("wpk", [C, 2 * D + C], F16, kind="ExternalInput").ap()
    # packed [bq | bk | ones(QC)]
    bpk_d = nc.dram_tensor("bpk", [1, 2 * D + QC], F16, kind="ExternalInput").ap()
    o_d = nc.dram_tensor("o", [Q, C], F16, kind="ExternalOutput").ap()

    with tile.TileContext(nc) as tc:
        with ExitStack() as ctx:
            big = ctx.enter_context(tc.tile_pool(name="big", bufs=24))
            const = ctx.enter_context(tc.tile_pool(name="const", bufs=1))
            ep = ctx.enter_context(tc.tile_pool(name="ep", bufs=4))
            ps = ctx.enter_context(tc.tile_pool(name="ps", bufs=2, space="PSUM"))
            po = ctx.enter_context(tc.tile_pool(name="po", bufs=4, space="PSUM"))

            # ---- constants / weights (3 triggers total) ----
            wpk_t = [const.tile([P, 2 * D + C], F16, tag=f"wpk{i}", name=f"wpk{i}")
                     for i in range(2)]
            for i in range(2):
                nc.scalar.dma_start(wpk_t[i][:], wpk_d[i * P:(i + 1) * P, :])
            bpk_t = const.tile([1, 2 * D + QC], F16, tag="bpk")
            nc.sync.dma_start(bpk_t[:], bpk_d)
            wq_sb = [wpk_t[i][:, 0:D] for i in range(2)]
            wk_sb = [wpk_t[i][:, D:2 * D] for i in range(2)]
            wv_sb = [wpk_t[i][:, 2 * D:] for i in range(2)]
            bq_sb = bpk_t[:, 0:D]
            bk_sb = bpk_t[:, D:2 * D]
            ones_sb = bpk_t[:, 2 * D:]

            # qrep: q^T replicated at partition blocks 0-31 and 32-63
            qrep = const.tile([2 * D, Q], F16, tag="qrep")
            # kT4: EVEN k tiles at partitions 0-31 (col (kt//2)*128),
            #      ODD  k tiles at partitions 32-63
            kT4 = const.tile([2 * D, 16 * P], F16, tag="kT4")
            kstage = const.tile([D, 16 * P], F16, tag="kstage")
            v_all = const.tile([P, KT * VW], F16, tag="vall")
            nc.vector.memset(
                v_all[:].rearrange("p (k c) -> p k c", c=VW)[:, :, C:C + 2], 1.0)

            # ---- x: two big const tiles per half, split across both HWDGE
            # queues (sync=SP, scalar=ACT; ACT is idle during the load) ----
            xkv_t = [const.tile([P, HW], F16, tag=f"xkv{i}", name=f"xkv{i}")
                     for i in range(2)]
            xq_t = [const.tile([P, Q], F16, tag=f"xq{i}", name=f"xq{i}")
                    for i in range(2)]
            QT4 = HW // 4
            # Queue layout: scalar (ACT) gets only the minimal chunk-0
            # pieces so exps can start early; sync carries the rest IN
            # CONSUMPTION ORDER, with quarters 2-3 deferred into proj_work
            # so the kT4/qrep staging DMAs are not stuck behind them.
            # All deferred triggers go on sync - never scalar (an x trigger
            # behind an exp in the ACT stream deadlocks: exp waits QK waits
            # x-data waits trigger waits exp).
            nc.scalar.dma_start(xkv_t[1][:, 0:QT4], xkv_d[P:2 * P, 0:QT4])
            nc.scalar.dma_start(xq_t[1][:, 0:QC], xq_d[P:2 * P, 0:QC])
            nc.sync.dma_start(xkv_t[0][:, 0:QT4], xkv_d[0:P, 0:QT4])
            nc.sync.dma_start(xq_t[0][:, 0:QC], xq_d[0:P, 0:QC])
            for i in range(2):
                nc.sync.dma_start(xkv_t[i][:, QT4:2 * QT4],
                                  xkv_d[i * P:(i + 1) * P, QT4:2 * QT4])
                nc.sync.dma_start(xq_t[i][:, QC:], xq_d[i * P:(i + 1) * P, QC:])

            def xdma(quarter):
                for i in range(2):
                    nc.sync.dma_start(
                        xkv_t[i][:, quarter * QT4:(quarter + 1) * QT4],
                        xkv_d[i * P:(i + 1) * P, quarter * QT4:(quarter + 1) * QT4])
            xq_sb = [[xq_t[i][:, j * QC:(j + 1) * QC] for j in range(Q // QC)]
                     for i in range(2)]
            xkv_sb = [[xkv_t[i][:, j * QC:(j + 1) * QC] for j in range(HW // QC)]
                      for i in range(2)]

            # ---- projections (PE, fp16 in / f32 psum) ----
            def kproj(j):
                kp = po.tile([D, QC], F32, tag="o", name="kp")
                nc.tensor.matmul(kp[:], wk_sb[0], xkv_sb[0][j],
                                 start=True, stop=not with_bias)
                nc.tensor.matmul(kp[:], wk_sb[1], xkv_sb[1][j],
                                 start=False, stop=not with_bias)
                if with_bias:
                    nc.tensor.matmul(kp[:], bk_sb, ones_sb, start=False, stop=True)
                # chunk j = k tiles 4j..4j+3: evens -> kT4[0:32], odds -> stage
                src = kp[:].rearrange("p (t c) -> p t c", c=P)
                dst_e = kT4[0:D, 2 * j * P:(2 * j + 2) * P].rearrange(
                    "p (t c) -> p t c", c=P)
                dst_o = kstage[:, 2 * j * P:(2 * j + 2) * P].rearrange(
                    "p (t c) -> p t c", c=P)
                nc.vector.tensor_copy(dst_o, src[:, 1::2])
                nc.vector.tensor_copy(dst_e, src[:, 0::2])
                nc.sync.dma_start(kT4[D:2 * D, 2 * j * P:(2 * j + 2) * P],
                                  kstage[:, 2 * j * P:(2 * j + 2) * P])

            def qproj(j):
                qp = po.tile([D, QC], F32, tag="o", name="qp")
                nc.tensor.matmul(qp[:], wq_sb[0], xq_sb[0][j],
                                 start=True, stop=not with_bias)
                nc.tensor.matmul(qp[:], wq_sb[1], xq_sb[1][j],
                                 start=False, stop=not with_bias)
                if with_bias:
                    nc.tensor.matmul(qp[:], bq_sb, ones_sb, start=False, stop=True)
                nc.vector.tensor_copy(qrep[0:D, j * QC:(j + 1) * QC], qp[:])
                nc.sync.dma_start(qrep[D:2 * D, j * QC:(j + 1) * QC],
                                  qrep[0:D, j * QC:(j + 1) * QC])

            v_sb = [v_all[:, t * VW:(t + 1) * VW] for t in range(KT)]

            def vproj(t):
                j, off = divmod(t, QC // P)
                vp = po.tile([P, C], F32, tag="o", name="vp")
                nc.tensor.matmul(
                    vp[:], xkv_sb[0][j][:, off * P:(off + 1) * P], wv_sb[0],
                    start=True, stop=False)
                nc.tensor.matmul(
                    vp[:], xkv_sb[1][j][:, off * P:(off + 1) * P], wv_sb[1],
                    start=False, stop=True)
                nc.vector.tensor_copy(v_sb[t][:, 0:C], vp[:])

            def vpair(g):
                vproj(2 * g)
                vproj(2 * g + 1)

            # chunk-0 deps first; the rest interleaves into the early
            # attention steps (all proj psum use ends before AV claims po).
            # With the even/odd pairing everything streams in natural order:
            # QK pair g needs kproj(g//2), AV pair g needs vpair(g).
            kproj(0)
            qproj(0)
            proj_work = [
                lambda: xdma(2), lambda: vpair(0), lambda: vpair(1),
                lambda: kproj(1), lambda: vpair(2),
                lambda: qproj(1), lambda: vpair(3),
                lambda: kproj(2), lambda: vpair(4), lambda: vpair(5),
                lambda: xdma(3),
                lambda: kproj(3), lambda: vpair(6),
                lambda: qproj(2), lambda: vpair(7),
                lambda: kproj(4), lambda: vpair(8), lambda: vpair(9),
                lambda: kproj(5), lambda: vpair(10),
                lambda: qproj(3), lambda: vpair(11),
                lambda: kproj(6), lambda: vpair(12), lambda: vpair(13),
                lambda: kproj(7), lambda: vpair(14), lambda: vpair(15),
            ]

            # ---- attention: flat 64-pair stream, AV lags QK by 2 pairs ----
            # pair (ci, g) = exp(scores) for k tiles (g, 16+g) of q chunk ci,
            # one [128, 1024] bf16 tile. AV of pair g covers kt=g and kt=16+g;
            # kt accumulation order [0,16,1,17,...] so start is kt==0 (pos 0)
            # and stop is kt==31 (pos 31).
            NP = NCH * 16
            AV_START = 18       # po banks stay proj-owned before this step

            def av_epilogue(ops, ci):
                for qs in range(QC // P):
                    op = ops[qs]
                    rinv = ep.tile([P, 1], F32, tag="rinv", name="rinv")
                    nc.vector.reciprocal(rinv[:], op[:, C:C + 1])
                    osb = ep.tile([P, C], F16, tag="osb", name="osb")
                    nc.vector.tensor_scalar_mul(osb[:], op[:, 0:C], rinv[:])
                    q0 = (ci * (QC // P) + qs) * P
                    nc.sync.dma_start(o_d[q0:q0 + P, :], osb[:])

            def av_pair(ops, pair_tile, g):
                for kt, half in ((2 * g, 0), (2 * g + 1, 1)):
                    for qs in range(QC // P):
                        nc.tensor.matmul(
                            ops[qs][:],
                            pair_tile[:, half * QC + qs * P: half * QC + (qs + 1) * P],
                            v_sb[kt][:],
                            start=(kt == 0), stop=(kt == KT - 1))

            pair_tiles = {}
            ops = {}
            av_done = 0
            step = 0
            wi = 0
            while av_done < NP:
                if step < NP:
                    ci, g = divmod(step, 16)
                    sc = ps.tile([P, 2 * QC], F32, tag="p", name="sc")
                    nc.tensor.matmul(
                        sc[:, 0:QC], kT4[0:D, g * P:(g + 1) * P],
                        qrep[0:D, ci * QC:(ci + 1) * QC],
                        start=True, stop=True, tile_position=(0, 0))
                    nc.tensor.matmul(
                        sc[:, QC:2 * QC], kT4[D:2 * D, g * P:(g + 1) * P],
                        qrep[D:2 * D, ci * QC:(ci + 1) * QC],
                        start=True, stop=True, tile_position=(D, 0))
                    Pt = big.tile([P, 2 * QC], BF16, tag="big", name="pt")
                    nc.scalar.activation(Pt[:], sc[:], EXP)
                    pair_tiles[step] = Pt
                for _ in range(2):
                    if wi < len(proj_work):
                        proj_work[wi]()
                        wi += 1
                budget = 2 if step >= AV_START else 0
                while budget > 0 and av_done < NP and av_done <= step - 2:
                    cav, gav = divmod(av_done, 16)
                    if gav == 0:
                        ops[cav] = [po.tile([P, VW], F32, tag="o", name="avo")
                                    for _ in range(QC // P)]
                    av_pair(ops[cav], pair_tiles.pop(av_done), gav)
                    if gav == 15:
                        av_epilogue(ops.pop(cav), cav)
                    av_done += 1
                    budget -= 1
                step += 1

    nc.compile()
    return nc


def _in_maps(x, Wq, bq, Wk, bk, Wv, bv):
    xf = np.ascontiguousarray(np.asarray(x, np.float32).reshape(B, C, HW)).astype(np.float16)
    wpk = np.concatenate([
        np.asarray(Wq, np.float32).T,
        np.asarray(Wk, np.float32).T,
        np.asarray(Wv, np.float32).T], axis=1).astype(np.float16)
    bpk = np.concatenate([
        np.asarray(bq, np.float32).reshape(1, D),
        np.asarray(bk, np.float32).reshape(1, D),
        np.ones((1, QC), np.float32)], axis=1).astype(np.float16)
    maps = []
    for core in range(NCORES):
        b, h = divmod(core, 2)
        maps.append({
            "xkv": xf[b],
            "xq": np.ascontiguousarray(xf[b][:, h * Q:(h + 1) * Q]),
            "wpk": np.ascontiguousarray(wpk),
            "bpk": np.ascontiguousarray(bpk),
        })
    return maps


def _gather(results, bv):
    out = np.empty((B, C, HW), np.float32)
    for core in range(NCORES):
        b, h = divmod(core, 2)
        out[b][:, h * Q:(h + 1) * Q] = results[core]["o"].T
    out += np.asarray(bv, np.float32).reshape(1, C, 1)
    return out.reshape(B, C, H, W)


def run(x, Wq, bq, Wk, bk, Wv, bv, **kwargs):
    with_bias = bool(np.any(np.asarray(bq)) or np.any(np.asarray(bk)))
    key = f"nc{int(with_bias)}"
    nc = _CACHE.get(key)
    if nc is None:
        nc = build_program(with_bias=with_bias)
        _CACHE[key] = nc
    maps = _in_maps(x, Wq, bq, Wk, bk, Wv, bv)
    res = run_bass_kernel_spmd(nc, maps, core_ids=list(range(NCORES)), **kwargs)
    return _gather(res.results, bv), res


def kernel(x, Wq, bq, Wk, bk, Wv, bv) -> np.ndarray:
    out, _ = run(x, Wq, bq, Wk, bk, Wv, bv)
    return out
